# revision 1
# baseline (speedup 1.0000x reference)
"""Trainium2 Bass kernel for nn_AntisymmetricLayer — v4 (PE-side reduction).

Same math as kernel.py, but the r-reduction and the lin add run on the
TensorEngine via accumulating matmuls against a 0/1 selection matrix, so the
VectorEngine does ONLY the elementwise products.

Layout trick: computation runs transposed. Per 512-token block:
  GpSimd   : z = x1-x2, s = x1+x2 on whole block [128, 512] bf16
  DMA xbar : transpose -> z^T, s^T [d, n-block] bf16
  PE       : A^T_c = P2_c^T @ z^T  [128kr, 512n] (8 chunks of kr), B^T_c same
             outT = W^T-matmul (lin, start) + sum_c sel_c^T @ prod_c (accum)
  ACT      : stage B^T_c PSUM -> SBUF bf16; evacuate outT -> SBUF
  DVE      : prod_c = A^T_c * B^T_c  (one PSUM + one SBUF operand)
  out in DRAM is [K, n_tokens]; host transposes during unshard.

sel_c[p, k] = 1 iff k == c*8 + p//16  (sums groups of 16 kr-partitions)
"""

import numpy as np
import ml_dtypes

import concourse.bass as bass
import concourse.mybir as mybir
import concourse.tile as tile
from concourse import bacc
from concourse.bass import ts
from concourse.bass_utils import run_bass_kernel_spmd

F32 = mybir.dt.float32
BF16 = mybir.dt.bfloat16

D = 128
K = 64
R = 16
KR = K * R  # 1024
NCHUNK = KR // 128  # 8 kr-chunks of 128
SELW = NCHUNK * 32  # 256 (32-wide strips)
CONST_W = 2 * KR + K + SELW + 2 * 256  # p2|q2|wt|sel|[I|I]|[-I|I] packed
N_CORES = 8
OUT_T = True  # DRAM output is [K, n]; host transposes
TILE = 128
CHUNK_TILES = 4     # tokens per block = 512
BLK = TILE * CHUNK_TILES


def build_bass(n_tokens: int = 16384):
    assert n_tokens % BLK == 0
    n_blocks = n_tokens // BLK

    nc = bacc.Bacc(None, target_bir_lowering=False)

    x1 = nc.declare_dram_parameter("x1", [n_tokens, D], F32, isOutput=False)
    x2 = nc.declare_dram_parameter("x2", [n_tokens, D], F32, isOutput=False)
    cw = nc.declare_dram_parameter("cw", [D, CONST_W], BF16, isOutput=False)
    # output stored transposed [K, n]; host transposes after gather
    out = nc.declare_dram_parameter("out", [K, n_tokens], F32, isOutput=True)

    with tile.TileContext(nc) as tc:
        with (
            tc.tile_pool(name="const", bufs=1) as cpool,
            tc.tile_pool(name="xin", bufs=3) as xpool,
            tc.tile_pool(name="zst", bufs=3) as ztpool,
            tc.tile_pool(name="bsp", bufs=4) as bspool,
            tc.tile_pool(name="prods", bufs=6) as ppool,
            tc.tile_pool(name="outs", bufs=3) as opool,
            tc.tile_pool(name="ptr", bufs=1, space="PSUM") as ptr_pool,
            tc.tile_pool(name="pa", bufs=2, space="PSUM") as pa_pool,
            tc.tile_pool(name="pb", bufs=2, space="PSUM") as pb_pool,
            tc.tile_pool(name="po", bufs=2, space="PSUM") as po_pool,
        ):
            cws = cpool.tile([D, CONST_W], BF16)
            nc.sync.dma_start(cws[:], cw[:])
            p2s = cws[:, 0:KR]
            q2s = cws[:, KR : 2 * KR]
            wts = cws[:, 2 * KR : 2 * KR + K]
            sels = cws[:, 2 * KR + K : 2 * KR + K + SELW]
            idpair = cws[:, 2 * KR + K + SELW : 2 * KR + K + SELW + 256]
            idpairn = cws[:, 2 * KR + K + SELW + 256 :]

            x1v = x1.rearrange("(c a p) d -> c p a d", p=TILE, a=CHUNK_TILES)
            x2v = x2.rearrange("(c a p) d -> c p a d", p=TILE, a=CHUNK_TILES)

            prev = None

            def do_tail(zt, st, j):
                # PE: lin first (opens the outT accumulation group),
                # then per-chunk A/B matmuls with sel-reduce skewed behind
                # NOTE: skip_group_check -- the CoreSim zero-region tracker
                # ignores the out base-partition, so the 32-row strip groups
                # false-positive. HW per-element has_written semantics are
                # exact: the full-width lin matmul (start=True) clears the
                # bank and sets bits for all 64 rows; strip matmuls accumulate.
                outp = po_pool.tile([K, BLK], F32, name=f"outp{j}", tag="outp")
                nc.tensor.matmul(outp[:], wts, zt[:], start=True, stop=False,
                                 skip_group_check=True)

                chunks = []  # (a_psum, prod_sb) pending sel-reduce

                def emit_sel(c, a_ps, b_sb):
                    prod = ppool.tile(
                        [128, BLK], BF16, name=f"prod{j}_{c}", tag="prod"
                    )
                    nc.vector.tensor_mul(prod[:], a_ps[:], b_sb[:])
                    # 32-row strip (c%2): consecutive chunks land on different
                    # col-groups and execute concurrently in the PE array
                    strip = outp[32 * (c % 2) : 32 * (c % 2) + 32, :]
                    nc.tensor.matmul(
                        strip,
                        sels[:, c * 32 : (c + 1) * 32],
                        prod[:],
                        start=False,
                        stop=(c >= NCHUNK - 2),
                        skip_group_check=True,
                    )

                for c in range(NCHUNK):
                    a = pa_pool.tile([128, BLK], F32, name=f"a{j}_{c}", tag="A")
                    nc.tensor.matmul(
                        a[:], p2s[:, ts(c, 128)], zt[:], start=True, stop=True
                    )
                    b = pb_pool.tile([128, BLK], F32, name=f"b{j}_{c}", tag="B")
                    nc.tensor.matmul(
                        b[:], q2s[:, ts(c, 128)], st[:], start=True, stop=True
                    )
                    bs = bspool.tile([128, BLK], BF16, name=f"bs{j}_{c}", tag="bs")
                    nc.scalar.copy(bs[:], b[:])
                    chunks.append((a, bs))
                    # emit sel-reduces in ADJACENT strip pairs so the two
                    # 32-row col-groups execute concurrently in the array
                    if c % 2 == 1:
                        emit_sel(c - 1, *chunks[c - 1])
                        emit_sel(c, *chunks[c])

                # ACT: evacuate outT, then DMA [K, 512] f32 (2KB rows)
                osb = opool.tile([K, BLK], F32, name=f"osb{j}", tag="osb")
                nc.scalar.copy(osb[:], outp[:])
                nc.sync.dma_start(out[:, ts(j, BLK)], osb[:])

            for j in range(n_blocks):
                x1c = xpool.tile([TILE, CHUNK_TILES, D], BF16, name=f"x1c{j}", tag="x1c")
                nc.gpsimd.dma_start(x1c[:], x1v[j])
                x2c = xpool.tile([TILE, CHUNK_TILES, D], BF16, name=f"x2c{j}", tag="x2c")
                nc.gpsimd.dma_start(x2c[:], x2v[j])

                # PE: z^T/s^T via paired transposing matmuls: stationary
                # x1_t streams [I|I] (writes x1^T to both pz_t and ps_t),
                # then x2_t streams [-I|I] accumulating -> pz_t|ps_t.
                # Layout [D, t, (pz|ps)]: 2 subtile-pairs per PSUM bank,
                # accumulation groups run sequentially per bank.
                pzs = ptr_pool.tile([D, 2 * BLK], F32, name=f"pzs{j}", tag="pzs")
                pzv = pzs.rearrange("p (t w) -> p t w", w=2 * TILE)
                for t in range(CHUNK_TILES):
                    pair = pzv[:, t, :]
                    nc.tensor.matmul(pair, x1c[:, t, :], idpair,
                                     start=True, stop=False)
                    nc.tensor.matmul(pair, x2c[:, t, :], idpairn,
                                     start=False, stop=True)

                # evacuate: zt on ACT, st on DVE (strided gather of the
                # per-t halves; inner 128 contiguous)
                zt = ztpool.tile([D, BLK], BF16, name=f"zt{j}", tag="zt")
                nc.scalar.copy(
                    zt.rearrange("p (t w) -> p t w", w=TILE),
                    pzv[:, :, 0:TILE],
                )
                st = ztpool.tile([D, BLK], BF16, name=f"st{j}", tag="st")
                nc.vector.tensor_copy(
                    st.rearrange("p (t w) -> p t w", w=TILE),
                    pzv[:, :, TILE : 2 * TILE],
                )

                if prev is not None:
                    do_tail(*prev)
                prev = (zt, st, j)

            do_tail(*prev)

    nc.finalize()
    return nc


def _perm():
    # out-row for k = 8c+t is  newk = 32*(c%2) + 8*(c//2) + t
    perm = np.zeros(K, dtype=np.int64)
    for c in range(NCHUNK):
        for t in range(8):
            perm[8 * c + t] = 32 * (c % 2) + 8 * (c // 2) + t
    return perm


def _make_sel():
    # sel_c maps kr-partition p to strip-local row 8*(c//2) + p//16
    sel = np.zeros((NCHUNK, 128, 32), dtype=np.float32)
    for c in range(NCHUNK):
        for p in range(128):
            sel[c, p, 8 * (c // 2) + p // 16] = 1.0
    return sel.transpose(1, 0, 2).reshape(128, NCHUNK * 32)


def _shard_and_pack(x1, x2, W_lin, P, Q):
    p2 = P.transpose(1, 0, 2).reshape(D, KR)
    q2 = Q.transpose(1, 0, 2).reshape(D, KR)
    wt = np.ascontiguousarray(W_lin.T)[:, np.argsort(_perm())]
    idp = np.eye(D, dtype=np.float32)
    idpair = np.concatenate([idp, idp], axis=1)
    idpairn = np.concatenate([-idp, idp], axis=1)
    cwv = np.concatenate([p2, q2, wt, _make_sel(), idpair, idpairn], axis=1).astype(
        ml_dtypes.bfloat16
    )
    assert cwv.shape == (D, CONST_W)

    in_maps = []
    for b in range(N_CORES):
        in_maps.append(
            {
                "x1": np.ascontiguousarray(x1[b]),
                "x2": np.ascontiguousarray(x2[b]),
                "cw": cwv,
            }
        )
    return in_maps


def postprocess(out_raw):
    """Per-core raw DRAM output [K, n] (permuted rows) -> [n, K] natural."""
    return np.ascontiguousarray(out_raw[_perm(), :].T)


def kernel(x1, x2, W_lin, P, Q):
    assert x1.shape == (N_CORES, 16384, D) and x2.shape == x1.shape
    nc = build_bass(16384)
    in_maps = _shard_and_pack(x1, x2, W_lin, P, Q)
    res = run_bass_kernel_spmd(nc, in_maps, core_ids=list(range(N_CORES)))
    out = np.stack(
        [postprocess(res.results[b]["out"]) for b in range(N_CORES)], axis=0
    )
    return out.astype(np.float32)



# revision 2
# speedup vs baseline: 1.0444x; 1.0444x over previous
"""Trainium2 Bass kernel for nn_AntisymmetricLayer — v5.1 (balanced pipeline).

Per 512-token block:
  DMA    : x1,x2 f32->bf16 token-major tiles [128, 4, 128]
  PE     : paired transposing matmuls -> pzs PSUM [d, (t, z|s, 128)]
  ACT    : zst: one FD=1024 copy pzs -> SBUF bf16 [d, (z|s, t, 128)]
           (so z^T = zst[:, :512], s^T = zst[:, 512:] are contiguous)
  PE     : per chunk-pair ph: A_c = P2_c^T z^T, B_c = Q2_c^T s^T into
           2-bank PSUM pair tiles [128, 1024]
  ACT    : bs_ph: FD=1024 copy B-pair -> SBUF bf16
  DVE    : prod_ph = A-pair(PSUM) * bs_ph(SBUF) -> SBUF bf16 [128, 1024]
  PE     : lin matmul (start=True over [128,512] outp bank) then 8 sel
           strip matmuls (32-wide, zero-padded cols, 4 col-groups) accumulate
  DVE    : osb: copy outp -> SBUF f32; DMA 64 used rows -> DRAM [64, n]

out row for k = 8c+t is 32*(c//2) + 8*(c%2) + t; host inverse-permutes.

PSUM: pa pool 2 slots x 2 banks (A-pairs); pb pool 2 slots x 2 banks shared
(tag-rotated) by pzs, 4 B-pairs, outp = 6 allocs/block.
"""

import numpy as np
import ml_dtypes

import concourse.bass as bass
import concourse.mybir as mybir
import concourse.tile as tile
from concourse import bacc
from concourse.bass import ts
from concourse.bass_utils import run_bass_kernel_spmd

F32 = mybir.dt.float32
BF16 = mybir.dt.bfloat16

D = 128
K = 64
R = 16
KR = K * R  # 1024
NCHUNK = KR // 128  # 8
TILE = 128
CT = 4  # token-tiles per block
BLK = TILE * CT  # 512
N_CORES = 8
SELW = NCHUNK * 32  # 256
# p2|q2|wt2|sel2|idpair|idpairn
CONST_W = 2 * KR + 128 + SELW + 2 * 256  # 2944


def build_bass(n_tokens: int = 16384):
    assert n_tokens % BLK == 0
    n_blocks = n_tokens // BLK

    nc = bacc.Bacc(None, target_bir_lowering=False)

    x1 = nc.declare_dram_parameter("x1", [n_tokens, D], F32, isOutput=False)
    x2 = nc.declare_dram_parameter("x2", [n_tokens, D], F32, isOutput=False)
    cw = nc.declare_dram_parameter("cw", [D, CONST_W], BF16, isOutput=False)
    # output stored permuted-transposed [64, n]; host fixes after gather
    out = nc.declare_dram_parameter("out", [K, n_tokens], F32, isOutput=True)
    outv = out.rearrange("(g p) n -> g p n", g=4)

    with tile.TileContext(nc) as tc:
        with (
            tc.tile_pool(name="const", bufs=1) as cpool,
            tc.tile_pool(name="xin", bufs=4) as xpool,
            tc.tile_pool(name="zst", bufs=3) as zpool,
            tc.tile_pool(name="bsp", bufs=6) as bspool,
            tc.tile_pool(name="prods", bufs=6) as ppool,
            tc.tile_pool(name="outs", bufs=4) as opool,
            tc.tile_pool(name="pa", bufs=2, space="PSUM") as pa_pool,
            tc.tile_pool(name="pb", bufs=2, space="PSUM") as pb_pool,
        ):
            cws = cpool.tile([D, CONST_W], BF16)
            nc.sync.dma_start(cws[:], cw[:])
            p2s = cws[:, 0:KR]
            q2s = cws[:, KR : 2 * KR]
            wt2s = cws[:, 2 * KR : 2 * KR + 128]
            sel2s = cws[:, 2 * KR + 128 : 2 * KR + 128 + SELW]
            idpair = cws[:, 2 * KR + 128 + SELW : 2 * KR + 128 + SELW + 256]
            idpairn = cws[:, 2 * KR + 128 + SELW + 256 :]

            x1v = x1.rearrange("(c a p) d -> c p a d", p=TILE, a=CT)
            x2v = x2.rearrange("(c a p) d -> c p a d", p=TILE, a=CT)

            xcs = {}
            zsts = {}
            prods = {}
            outps = {}

            def emit_dma(j):
                if j >= n_blocks:
                    return
                x1c = xpool.tile([TILE, CT, D], BF16, name=f"x1c{j}", tag="x1c")
                nc.gpsimd.dma_start(x1c[:], x1v[j])
                x2c = xpool.tile([TILE, CT, D], BF16, name=f"x2c{j}", tag="x2c")
                nc.gpsimd.dma_start(x2c[:], x2v[j])
                xcs[j] = (x1c, x2c)

            def emit_transpose(j):
                # pzs layout [d, (t, zs, 128)]; zst layout [d, (zs, t, 128)]
                if j >= n_blocks:
                    return
                x1c, x2c = xcs[j]
                pzs = pb_pool.tile([D, 2 * BLK], F32, name=f"pzs{j}", tag="pb")
                for t in range(CT):
                    pair = pzs[:, ts(t, 256)]
                    nc.tensor.matmul(pair, x1c[:, t, :], idpair,
                                     start=True, stop=False)
                    nc.tensor.matmul(pair, x2c[:, t, :], idpairn,
                                     start=False, stop=True)
                zst = zpool.tile([D, 2 * BLK], BF16, name=f"zst{j}", tag="zst")
                nc.scalar.copy(
                    zst.rearrange("p (c t w) -> p c t w", c=2, t=CT),
                    pzs.rearrange("p (t c w) -> p c t w", t=CT, c=2),
                )
                zsts[j] = zst

            def emit_phase(j, ph):
                zst = zsts[j]
                ztv = zst[:, 0:BLK]
                stv = zst[:, BLK : 2 * BLK]
                pb = pb_pool.tile([D, 2 * BLK], F32, name=f"pb{j}_{ph}", tag="pb")
                pa = pa_pool.tile([D, 2 * BLK], F32, name=f"pa{j}_{ph}", tag="pa")
                for h in range(2):
                    c = 2 * ph + h
                    nc.tensor.matmul(
                        pb[:, ts(h, BLK)], q2s[:, ts(c, 128)], stv,
                        start=True, stop=True,
                    )
                    nc.tensor.matmul(
                        pa[:, ts(h, BLK)], p2s[:, ts(c, 128)], ztv,
                        start=True, stop=True,
                    )
                bs = bspool.tile([D, 2 * BLK], BF16, name=f"bs{j}_{ph}", tag="bs")
                nc.scalar.copy(bs[:], pb[:])
                pr = ppool.tile([D, 2 * BLK], BF16, name=f"pr{j}_{ph}", tag="pr")
                nc.vector.tensor_mul(pr[:], pa[:], bs[:])
                prods.setdefault(j, []).append(pr)

            def emit_sel(j, c):
                op = outps[j][:, 0:BLK]
                g = c // 2
                strip = op[32 * g : 32 * g + 32, :]
                nc.tensor.matmul(
                    strip,
                    sel2s[:, ts(c, 32)],
                    prods[j][c // 2][:, ts(c % 2, BLK)],
                    start=False,
                    stop=(c == 7),
                    skip_group_check=True,
                    tile_position=(0, 32 * g),
                )

            def emit_lin(j):
                outp = pa_pool.tile([D, 2 * BLK], F32, name=f"outp{j}", tag="pa")
                outps[j] = outp
                nc.tensor.matmul(outp[:, 0:BLK], wt2s, zsts[j][:, 0:BLK],
                                 start=True, stop=False, skip_group_check=True)

            def emit_tail(j):
                # evacuation of block j
                osb = opool.tile([D, BLK], F32, name=f"osb{j}", tag="osb")
                nc.vector.tensor_copy(osb[:], outps[j][:, 0:BLK])
                for g in range(4):
                    nc.sync.dma_start(
                        outv[g, :, ts(j, BLK)], osb[32 * g : 32 * g + 16, :]
                    )
                del outps[j], prods[j], zsts[j]

            emit_dma(0)
            emit_dma(1)
            for j in range(n_blocks):
                emit_dma(j + 2)
                emit_transpose(j)
                for ph in range(4):
                    emit_phase(j, ph)
                emit_lin(j)
                for c in (0, 2, 4, 6, 1, 3, 5, 7):
                    emit_sel(j, c)
                emit_tail(j)

    nc.finalize()
    return nc


def _perm():
    # out-row for k = 8c+t is 32*(c//2) + 8*(c%2) + t
    perm = np.zeros(K, dtype=np.int64)
    for c in range(NCHUNK):
        for t in range(8):
            perm[8 * c + t] = 32 * (c // 2) + 8 * (c % 2) + t
    return perm


def _make_sel():
    # chunk c stationary cols [32c, 32c+32): col 8*(c%2)+t sums partitions
    # [16t, 16t+16); other cols zero
    sel = np.zeros((128, NCHUNK * 32), dtype=np.float32)
    for c in range(NCHUNK):
        for t in range(8):
            sel[16 * t : 16 * t + 16, 32 * c + 8 * (c % 2) + t] = 1.0
    return sel


def _make_wt2(W_lin):
    wt2 = np.zeros((D, 128), dtype=np.float32)
    perm = _perm()
    for k in range(K):
        wt2[:, perm[k]] = W_lin[k, :]
    return wt2


def _shard_and_pack(x1, x2, W_lin, P, Q):
    p2 = P.transpose(1, 0, 2).reshape(D, KR)
    q2 = Q.transpose(1, 0, 2).reshape(D, KR)
    idp = np.eye(D, dtype=np.float32)
    idpair = np.concatenate([idp, idp], axis=1)
    idpairn = np.concatenate([-idp, idp], axis=1)
    cwv = np.concatenate(
        [p2, q2, _make_wt2(W_lin), _make_sel(), idpair, idpairn], axis=1
    ).astype(ml_dtypes.bfloat16)
    assert cwv.shape == (D, CONST_W)

    in_maps = []
    for b in range(N_CORES):
        in_maps.append(
            {
                "x1": np.ascontiguousarray(x1[b]),
                "x2": np.ascontiguousarray(x2[b]),
                "cw": cwv,
            }
        )
    return in_maps


def postprocess(out_raw):
    """Per-core raw DRAM output [64, n] -> [n, K].

    DRAM row for k = 8c+t is 16*(c//2) + 8*(c%2) + t = k (the strided
    evacuation DMA already compacts the 32-row strips), so no permutation.
    """
    return np.ascontiguousarray(out_raw.T)


def kernel(x1, x2, W_lin, P, Q):
    assert x1.shape == (N_CORES, 16384, D) and x2.shape == x1.shape
    nc = build_bass(16384)
    in_maps = _shard_and_pack(x1, x2, W_lin, P, Q)
    res = run_bass_kernel_spmd(nc, in_maps, core_ids=list(range(N_CORES)))
    out = np.stack(
        [postprocess(res.results[b]["out"]) for b in range(N_CORES)], axis=0
    )
    return out.astype(np.float32)


# revision 4
# speedup vs baseline: 1.0461x; 1.0016x over previous
"""Trainium2 Bass kernel for nn_AntisymmetricLayer — v5 (balanced pipeline).

Per 512-token block:
  DMA    : x1,x2 f32->bf16 token-major tiles [128, 4, 128]
  PE     : paired transposing matmuls -> pzs PSUM [d, (t, z|s, 128)]
  ACT    : zst: one FD=1024 copy pzs -> SBUF bf16 [d, (z|s, t, 128)]
           (so z^T = zst[:, :512], s^T = zst[:, 512:] are contiguous)
  PE     : per chunk-pair ph: A_c = P2_c^T z^T, B_c = Q2_c^T s^T into
           2-bank PSUM pair tiles [128, 1024]
  ACT    : bs_ph: FD=1024 copy B-pair -> SBUF bf16
  DVE    : prod_ph = A-pair(PSUM) * bs_ph(SBUF) -> SBUF bf16 [128, 1024]
  PE     : lin matmul (start=True over [128,512] outp bank) then 8 sel
           strip matmuls (32-wide, zero-padded cols, 4 col-groups) accumulate
  DVE    : osb: copy outp -> SBUF f32; DMA 64 used rows -> DRAM [64, n]

out row for k = 8c+t is 32*(c//2) + 8*(c%2) + t; host inverse-permutes.

PSUM: pa pool 2 slots x 2 banks (A-pairs); pb pool 2 slots x 2 banks shared
(tag-rotated) by pzs, 4 B-pairs, outp = 6 allocs/block.
"""

import numpy as np
import ml_dtypes

import concourse.bass as bass
import concourse.mybir as mybir
import concourse.tile as tile
from concourse import bacc
from concourse.bass import ts
from concourse.bass_utils import run_bass_kernel_spmd

F32 = mybir.dt.float32
BF16 = mybir.dt.bfloat16

D = 128
K = 64
R = 16
KR = K * R  # 1024
NCHUNK = KR // 128  # 8
TILE = 128
CT = 4  # token-tiles per block
BLK = TILE * CT  # 512
N_CORES = 8
SELW = NCHUNK * 32  # 256
# p2|q2|wt2|sel2|idpair|idpairn
CONST_W = 2 * KR + 128 + SELW + 2 * 256  # 2944


def build_bass(n_tokens: int = 16384):
    assert n_tokens % BLK == 0
    n_blocks = n_tokens // BLK

    nc = bacc.Bacc(None, target_bir_lowering=False)

    x1 = nc.declare_dram_parameter("x1", [n_tokens, D], F32, isOutput=False)
    x2 = nc.declare_dram_parameter("x2", [n_tokens, D], F32, isOutput=False)
    cw = nc.declare_dram_parameter("cw", [D, CONST_W], BF16, isOutput=False)
    # output stored permuted-transposed [64, n]; host fixes after gather
    out = nc.declare_dram_parameter("out", [K, n_tokens], F32, isOutput=True)
    outv = out.rearrange("(g p) n -> g p n", g=4)

    with tile.TileContext(nc) as tc:
        with (
            tc.tile_pool(name="const", bufs=1) as cpool,
            tc.tile_pool(name="xin", bufs=4) as xpool,
            tc.tile_pool(name="zst", bufs=3) as zpool,
            tc.tile_pool(name="bsp", bufs=6) as bspool,
            tc.tile_pool(name="prods", bufs=6) as ppool,
            tc.tile_pool(name="outs", bufs=4) as opool,
            tc.tile_pool(name="pa", bufs=2, space="PSUM") as pa_pool,
            tc.tile_pool(name="pb", bufs=2, space="PSUM") as pb_pool,
        ):
            cws = cpool.tile([D, CONST_W], BF16)
            nc.sync.dma_start(cws[:, 0:512], cw[:, 0:512])
            nc.sync.dma_start(cws[:, 512:], cw[:, 512:])
            idpair = cws[:, 0:256]
            idpairn = cws[:, 256:512]
            p2s = cws[:, 512 : 512 + KR]
            q2s = cws[:, 512 + KR : 512 + 2 * KR]
            wt2s = cws[:, 512 + 2 * KR : 512 + 2 * KR + 128]
            sel2s = cws[:, 512 + 2 * KR + 128 : 512 + 2 * KR + 128 + SELW]

            x1v = x1.rearrange("(c a p) d -> c p a d", p=TILE, a=CT)
            x2v = x2.rearrange("(c a p) d -> c p a d", p=TILE, a=CT)

            # PE warm-up: ~2.6us of back-to-back dummy matmuls right after
            # the first const DMA lands, so the HAM clock-gate reaches 8/8
            # before the real pipeline fills.
            warm = pb_pool.tile([D, 2 * BLK], F32, name="warm", tag="pb")
            for _ in range(24):
                nc.tensor.matmul(warm[:, 0:256], idpair[:, 0:128], idpair,
                                 start=True, stop=True)

            xcs = {}
            zsts = {}
            prods = {}
            outps = {}

            def emit_dma(j):
                if j >= n_blocks:
                    return
                x1c = xpool.tile([TILE, CT, D], BF16, name=f"x1c{j}", tag="x1c")
                nc.gpsimd.dma_start(x1c[:], x1v[j])
                x2c = xpool.tile([TILE, CT, D], BF16, name=f"x2c{j}", tag="x2c")
                nc.gpsimd.dma_start(x2c[:], x2v[j])
                xcs[j] = (x1c, x2c)

            def emit_transpose(j):
                # pzs layout [d, (t, zs, 128)]; zst layout [d, (zs, t, 128)]
                if j >= n_blocks:
                    return
                x1c, x2c = xcs[j]
                pzs = pb_pool.tile([D, 2 * BLK], F32, name=f"pzs{j}", tag="pb")
                for t in range(CT):
                    pair = pzs[:, ts(t, 256)]
                    nc.tensor.matmul(pair, x1c[:, t, :], idpair,
                                     start=True, stop=False)
                    nc.tensor.matmul(pair, x2c[:, t, :], idpairn,
                                     start=False, stop=True)
                zst = zpool.tile([D, 2 * BLK], BF16, name=f"zst{j}", tag="zst")
                nc.scalar.copy(
                    zst.rearrange("p (c t w) -> p c t w", c=2, t=CT),
                    pzs.rearrange("p (t c w) -> p c t w", t=CT, c=2),
                )
                zsts[j] = zst

            def emit_phase(j, ph):
                zst = zsts[j]
                ztv = zst[:, 0:BLK]
                stv = zst[:, BLK : 2 * BLK]
                pb = pb_pool.tile([D, 2 * BLK], F32, name=f"pb{j}_{ph}", tag="pb")
                pa = pa_pool.tile([D, 2 * BLK], F32, name=f"pa{j}_{ph}", tag="pa")
                for h in range(2):
                    nc.tensor.matmul(
                        pb[:, ts(h, BLK)], q2s[:, ts(2 * ph + h, 128)], stv,
                        start=True, stop=True,
                    )
                for h in range(2):
                    nc.tensor.matmul(
                        pa[:, ts(h, BLK)], p2s[:, ts(2 * ph + h, 128)], ztv,
                        start=True, stop=True,
                    )
                bs = bspool.tile([D, 2 * BLK], BF16, name=f"bs{j}_{ph}", tag="bs")
                nc.scalar.copy(bs[:], pb[:])
                pr = ppool.tile([D, 2 * BLK], BF16, name=f"pr{j}_{ph}", tag="pr")
                nc.vector.tensor_mul(pr[:], pa[:], bs[:])
                prods.setdefault(j, []).append(pr)

            def emit_sel(j, c):
                op = outps[j][:, 0:BLK]
                g = c // 2
                strip = op[32 * g : 32 * g + 32, :]
                nc.tensor.matmul(
                    strip,
                    sel2s[:, ts(c, 32)],
                    prods[j][c // 2][:, ts(c % 2, BLK)],
                    start=False,
                    stop=(c == 7),
                    skip_group_check=True,
                    tile_position=(0, 32 * g),
                )

            def emit_lin(j):
                outp = pa_pool.tile([D, 2 * BLK], F32, name=f"outp{j}", tag="pa")
                outps[j] = outp
                nc.tensor.matmul(outp[:, 0:BLK], wt2s, zsts[j][:, 0:BLK],
                                 start=True, stop=False, skip_group_check=True)

            def emit_tail(j):
                # evacuation of block j
                osb = opool.tile([D, BLK], F32, name=f"osb{j}", tag="osb")
                nc.scalar.copy(osb[:], outps[j][:, 0:BLK])
                for g in range(4):
                    nc.sync.dma_start(
                        outv[g, :, ts(j, BLK)], osb[32 * g : 32 * g + 16, :]
                    )
                del outps[j], prods[j], zsts[j]

            emit_dma(0)
            emit_dma(1)
            emit_transpose(0)
            for j in range(n_blocks):
                emit_dma(j + 2)
                emit_phase(j, 0)
                emit_phase(j, 1)
                emit_transpose(j + 1)
                emit_phase(j, 2)
                emit_phase(j, 3)
                emit_lin(j)
                for c in (0, 2, 4, 6, 1, 3, 5, 7):
                    emit_sel(j, c)
                emit_tail(j)

    nc.finalize()
    return nc


def _perm():
    # out-row for k = 8c+t is 32*(c//2) + 8*(c%2) + t
    perm = np.zeros(K, dtype=np.int64)
    for c in range(NCHUNK):
        for t in range(8):
            perm[8 * c + t] = 32 * (c // 2) + 8 * (c % 2) + t
    return perm


def _make_sel():
    # chunk c stationary cols [32c, 32c+32): col 8*(c%2)+t sums partitions
    # [16t, 16t+16); other cols zero
    sel = np.zeros((128, NCHUNK * 32), dtype=np.float32)
    for c in range(NCHUNK):
        for t in range(8):
            sel[16 * t : 16 * t + 16, 32 * c + 8 * (c % 2) + t] = 1.0
    return sel


def _make_wt2(W_lin):
    wt2 = np.zeros((D, 128), dtype=np.float32)
    perm = _perm()
    for k in range(K):
        wt2[:, perm[k]] = W_lin[k, :]
    return wt2


def _shard_and_pack(x1, x2, W_lin, P, Q):
    p2 = P.transpose(1, 0, 2).reshape(D, KR)
    q2 = Q.transpose(1, 0, 2).reshape(D, KR)
    idp = np.eye(D, dtype=np.float32)
    idpair = np.concatenate([idp, idp], axis=1)
    idpairn = np.concatenate([-idp, idp], axis=1)
    cwv = np.concatenate(
        [idpair, idpairn, p2, q2, _make_wt2(W_lin), _make_sel()], axis=1
    ).astype(ml_dtypes.bfloat16)
    assert cwv.shape == (D, CONST_W)

    in_maps = []
    for b in range(N_CORES):
        in_maps.append(
            {
                "x1": np.ascontiguousarray(x1[b]),
                "x2": np.ascontiguousarray(x2[b]),
                "cw": cwv,
            }
        )
    return in_maps


def postprocess(out_raw):
    """Per-core raw DRAM output [64, n] -> [n, K].

    DRAM row for k = 8c+t is 16*(c//2) + 8*(c%2) + t = k (the strided
    evacuation DMA already compacts the 32-row strips), so no permutation.
    """
    return np.ascontiguousarray(out_raw.T)


def kernel(x1, x2, W_lin, P, Q):
    assert x1.shape == (N_CORES, 16384, D) and x2.shape == x1.shape
    nc = build_bass(16384)
    in_maps = _shard_and_pack(x1, x2, W_lin, P, Q)
    res = run_bass_kernel_spmd(nc, in_maps, core_ids=list(range(N_CORES)))
    out = np.stack(
        [postprocess(res.results[b]["out"]) for b in range(N_CORES)], axis=0
    )
    return out.astype(np.float32)


# revision 5
# speedup vs baseline: 1.0801x; 1.0325x over previous
"""Trainium2 Bass kernel for nn_AntisymmetricLayer — v5 (balanced pipeline).

Per 512-token block:
  DMA    : x1,x2 f32->bf16 token-major tiles [128, 4, 128]
  PE     : paired transposing matmuls -> pzs PSUM [d, (t, z|s, 128)]
  ACT    : zst: one FD=1024 copy pzs -> SBUF bf16 [d, (z|s, t, 128)]
           (so z^T = zst[:, :512], s^T = zst[:, 512:] are contiguous)
  PE     : per chunk-pair ph: A_c = P2_c^T z^T, B_c = Q2_c^T s^T into
           2-bank PSUM pair tiles [128, 1024]
  ACT    : bs_ph: FD=1024 copy B-pair -> SBUF bf16
  DVE    : prod_ph = A-pair(PSUM) * bs_ph(SBUF) -> SBUF bf16 [128, 1024]
  PE     : lin matmul (start=True over [128,512] outp bank) then 8 sel
           strip matmuls (32-wide, zero-padded cols, 4 col-groups) accumulate
  DVE    : osb: copy outp -> SBUF f32; DMA 64 used rows -> DRAM [64, n]

out row for k = 8c+t is 32*(c//2) + 8*(c%2) + t; host inverse-permutes.

PSUM: pa pool 2 slots x 2 banks (A-pairs); pb pool 2 slots x 2 banks shared
(tag-rotated) by pzs, 4 B-pairs, outp = 6 allocs/block.
"""

import numpy as np
import ml_dtypes

import concourse.bass as bass
import concourse.mybir as mybir
import concourse.tile as tile
from concourse import bacc
from concourse.bass import ts
from concourse.bass_utils import run_bass_kernel_spmd

F32 = mybir.dt.float32
BF16 = mybir.dt.bfloat16

D = 128
K = 64
R = 16
KR = K * R  # 1024
NCHUNK = KR // 128  # 8
TILE = 128
CT = 4  # token-tiles per block
BLK = TILE * CT  # 512
N_CORES = 8
SELW = NCHUNK * 32  # 256
# p2|q2|wt2|sel2|idpair|idpairn
CONST_W = 2 * KR + 128 + SELW + 2 * 256  # 2944


def build_bass(n_tokens: int = 16384):
    assert n_tokens % BLK == 0
    n_blocks = n_tokens // BLK

    nc = bacc.Bacc(None, target_bir_lowering=False)

    x1 = nc.declare_dram_parameter("x1", [n_tokens, D], F32, isOutput=False)
    x2 = nc.declare_dram_parameter("x2", [n_tokens, D], F32, isOutput=False)
    cw = nc.declare_dram_parameter("cw", [D, CONST_W], BF16, isOutput=False)
    # output stored permuted-transposed [64, n]; host fixes after gather
    out = nc.declare_dram_parameter("out", [K, n_tokens], F32, isOutput=True)
    outv = out.rearrange("(g p) n -> g p n", g=4)

    with tile.TileContext(nc) as tc:
        with (
            tc.tile_pool(name="const", bufs=1) as cpool,
            tc.tile_pool(name="xin", bufs=4) as xpool,
            tc.tile_pool(name="zst", bufs=3) as zpool,
            tc.tile_pool(name="bsp", bufs=6) as bspool,
            tc.tile_pool(name="prods", bufs=6) as ppool,
            tc.tile_pool(name="outs", bufs=4) as opool,
            tc.tile_pool(name="pa", bufs=2, space="PSUM") as pa_pool,
            tc.tile_pool(name="pb", bufs=2, space="PSUM") as pb_pool,
        ):
            cws = cpool.tile([D, CONST_W], BF16)
            nc.sync.dma_start(cws[:, 0:512], cw[:, 0:512])
            nc.sync.dma_start(cws[:, 512:], cw[:, 512:])
            idpair = cws[:, 0:256]
            idpairn = cws[:, 256:512]
            p2s = cws[:, 512 : 512 + KR]
            q2s = cws[:, 512 + KR : 512 + 2 * KR]
            wt2s = cws[:, 512 + 2 * KR : 512 + 2 * KR + 128]
            sel2s = cws[:, 512 + 2 * KR + 128 : 512 + 2 * KR + 128 + SELW]

            x1v = x1.rearrange("(c a p) d -> c p a d", p=TILE, a=CT)
            x2v = x2.rearrange("(c a p) d -> c p a d", p=TILE, a=CT)

            # PE warm-up: ~2.6us of back-to-back dummy matmuls right after
            # the first const DMA lands, so the HAM clock-gate reaches 8/8
            # before the real pipeline fills.
            warm = pb_pool.tile([D, 2 * BLK], F32, name="warm", tag="pb")
            for _ in range(24):
                nc.tensor.matmul(warm[:, 0:256], idpair[:, 0:128], idpair,
                                 start=True, stop=True)

            xcs = {}
            zsts = {}
            prods = {}
            outps = {}

            def emit_dma(j):
                if j >= n_blocks:
                    return
                x1c = xpool.tile([TILE, CT, D], BF16, name=f"x1c{j}", tag="x1c")
                nc.gpsimd.dma_start(x1c[:], x1v[j])
                x2c = xpool.tile([TILE, CT, D], BF16, name=f"x2c{j}", tag="x2c")
                nc.gpsimd.dma_start(x2c[:], x2v[j])
                xcs[j] = (x1c, x2c)

            def emit_transpose(j):
                # pzs layout [d, (t, zs, 128)]; zst layout [d, (zs, t, 128)]
                if j >= n_blocks:
                    return
                x1c, x2c = xcs[j]
                pzs = pb_pool.tile([D, 2 * BLK], F32, name=f"pzs{j}", tag="pb")
                for t in range(CT):
                    pair = pzs[:, ts(t, 256)]
                    nc.tensor.matmul(pair, x1c[:, t, :], idpair,
                                     start=True, stop=False)
                    nc.tensor.matmul(pair, x2c[:, t, :], idpairn,
                                     start=False, stop=True)
                zst = zpool.tile([D, 2 * BLK], BF16, name=f"zst{j}", tag="zst")
                nc.scalar.copy(
                    zst.rearrange("p (c t w) -> p c t w", c=2, t=CT),
                    pzs.rearrange("p (t c w) -> p c t w", t=CT, c=2),
                )
                zsts[j] = zst

            def emit_phase(j, ph):
                zst = zsts[j]
                ztv = zst[:, 0:BLK]
                stv = zst[:, BLK : 2 * BLK]
                pb = pb_pool.tile([D, 2 * BLK], F32, name=f"pb{j}_{ph}", tag="pb")
                pa = pa_pool.tile([D, 2 * BLK], F32, name=f"pa{j}_{ph}", tag="pa")
                for h in range(2):
                    nc.tensor.matmul(
                        pb[:, ts(h, BLK)], q2s[:, ts(2 * ph + h, 128)], stv,
                        start=True, stop=True,
                    )
                for h in range(2):
                    nc.tensor.matmul(
                        pa[:, ts(h, BLK)], p2s[:, ts(2 * ph + h, 128)], ztv,
                        start=True, stop=True,
                    )
                bs = bspool.tile([D, 2 * BLK], BF16, name=f"bs{j}_{ph}", tag="bs")
                nc.scalar.copy(bs[:], pb[:])
                pr = ppool.tile([D, 2 * BLK], BF16, name=f"pr{j}_{ph}", tag="pr")
                nc.vector.tensor_mul(pr[:], pa[:], bs[:])
                prods.setdefault(j, []).append(pr)

            def emit_sel(j, c):
                op = outps[j][:, 0:BLK]
                g = c // 2
                strip = op[32 * g : 32 * g + 32, :]
                nc.tensor.matmul(
                    strip,
                    sel2s[:, ts(c, 32)],
                    prods[j][c // 2][:, ts(c % 2, BLK)],
                    start=False,
                    stop=(c == 7),
                    skip_group_check=True,
                    tile_position=(0, 32 * g),
                )

            def emit_lin(j):
                outp = pa_pool.tile([D, 2 * BLK], F32, name=f"outp{j}", tag="pa")
                outps[j] = outp
                nc.tensor.matmul(outp[:, 0:BLK], wt2s, zsts[j][:, 0:BLK],
                                 start=True, stop=False, skip_group_check=True)

            def emit_tail(j):
                # evacuation of block j
                osb = opool.tile([D, BLK], F32, name=f"osb{j}", tag="osb")
                nc.vector.tensor_copy(osb[:], outps[j][:, 0:BLK])
                for g in range(4):
                    nc.sync.dma_start(
                        outv[g, :, ts(j, BLK)], osb[32 * g : 32 * g + 16, :]
                    )
                del outps[j], prods[j], zsts[j]

            emit_dma(0)
            emit_dma(1)
            emit_transpose(0)
            for j in range(n_blocks):
                emit_dma(j + 2)
                emit_phase(j, 0)
                emit_phase(j, 1)
                emit_transpose(j + 1)
                emit_phase(j, 2)
                emit_phase(j, 3)
                emit_lin(j)
                for c in (0, 2, 4, 6, 1, 3, 5, 7):
                    emit_sel(j, c)
                emit_tail(j)

    nc.finalize()
    return nc


def _perm():
    # out-row for k = 8c+t is 32*(c//2) + 8*(c%2) + t
    perm = np.zeros(K, dtype=np.int64)
    for c in range(NCHUNK):
        for t in range(8):
            perm[8 * c + t] = 32 * (c // 2) + 8 * (c % 2) + t
    return perm


def _make_sel():
    # chunk c stationary cols [32c, 32c+32): col 8*(c%2)+t sums partitions
    # [16t, 16t+16); other cols zero
    sel = np.zeros((128, NCHUNK * 32), dtype=np.float32)
    for c in range(NCHUNK):
        for t in range(8):
            sel[16 * t : 16 * t + 16, 32 * c + 8 * (c % 2) + t] = 1.0
    return sel


def _make_wt2(W_lin):
    wt2 = np.zeros((D, 128), dtype=np.float32)
    perm = _perm()
    for k in range(K):
        wt2[:, perm[k]] = W_lin[k, :]
    return wt2


def _shard_and_pack(x1, x2, W_lin, P, Q):
    p2 = P.transpose(1, 0, 2).reshape(D, KR)
    q2 = Q.transpose(1, 0, 2).reshape(D, KR)
    idp = np.eye(D, dtype=np.float32)
    idpair = np.concatenate([idp, idp], axis=1)
    idpairn = np.concatenate([-idp, idp], axis=1)
    cwv = np.concatenate(
        [idpair, idpairn, p2, q2, _make_wt2(W_lin), _make_sel()], axis=1
    ).astype(ml_dtypes.bfloat16)
    assert cwv.shape == (D, CONST_W)

    in_maps = []
    for b in range(N_CORES):
        in_maps.append(
            {
                "x1": np.ascontiguousarray(x1[b]),
                "x2": np.ascontiguousarray(x2[b]),
                "cw": cwv,
            }
        )
    return in_maps


def postprocess(out_raw):
    """Per-core raw DRAM output [64, n] -> [n, K].

    DRAM row for k = 8c+t is 16*(c//2) + 8*(c%2) + t = k (the strided
    evacuation DMA already compacts the 32-row strips), so no permutation.
    """
    return np.ascontiguousarray(out_raw.T)


def kernel(x1, x2, W_lin, P, Q):
    assert x1.shape == (N_CORES, 16384, D) and x2.shape == x1.shape
    nc = build_bass(16384)
    in_maps = _shard_and_pack(x1, x2, W_lin, P, Q)
    res = run_bass_kernel_spmd(nc, in_maps, core_ids=list(range(N_CORES)))
    out = np.stack(
        [postprocess(res.results[b]["out"]) for b in range(N_CORES)], axis=0
    )
    return out.astype(np.float32)
